# revision 4
# baseline (speedup 1.0000x reference)
"""Causal centroid pyramid + phase transport, Bass/Tile kernel for 8 TRN2 cores.

Problem (hardcoded): x (4, 4096, 512) fp32 -> out (4, 4096, 8, 512) fp32.

Math: for scale j (W = 2^j), with mu_0 = x, mu_{j+1} = 0.5*(mu_j + shift_W(mu_j)):
  d_j = phase_transport(mu_j, shift_W(mu_j)) with position masks.
The transport collapses to y_j = s0*S_j + s1*S_{j+1} with unscaled dyadic sums
S_j = 2^j*mu_j and per-token scalars from nu2_j=|S_j|^2, nv2_j=shift(nu2_j),
P_j=(nu2_{j+1}-nu2_j-nv2_j)/2 (tA,tB are scale-invariant so the 2^-j folds
into the host-precomputed masks scm/m1: s0=(tA-tB+2)*scm+m1, s1=(tB-1)*scm).

v3 design (vs v1 baseline, informed by walrus/HW legality probes):
- S pyramid in bf16 (rel err ~4e-3 vs the 2e-2 gate; halves shift-DMA bytes).
- Per level, S_{j+1} is built either on the PE (banded bf16 matmuls A_j@S[i] +
  F_j@S[i-1] -> PSUM -> ACT copy to SBUF) or via SBUF->SBUF shift DMA + Pool
  (gpsimd) adds: PE_MASK selects; default puts level 0 on the PE and 6 on
  DMA (adds rotate Pool/Pool/DVE) so ACT/DVE keep capacity for reduces and y.
- Level 8 needs no shift DMA or matmul: S_8[i] = S_7[i] + S_7[i-1] is a free
  column-offset pair, materialized at the tail by Pool adds (Pool/ACT are
  otherwise idle there) with nu2_8 from ACT squares.
- Reduces: ACT Square+accum (PSUM or SBUF) and a custom DVE dot-accum op
  (InstTensorTensorReduce crashes this runtime; GPSIMD cannot reduce or touch
  PSUM; DVE may read at most one PSUM operand).
- y tiles emit bf16 via a custom DVE op (host upcast).

Sharding: 8 cores = (batch b in 0..3) x (sequence half h in 0..1). Each core
processes 2048 output tokens plus a 256-token lookback halo (recomputed).
"""

import os
import numpy as np
from contextlib import ExitStack

import concourse.bass as bass
import concourse.tile as tile
from concourse import bacc, mybir
from concourse.bass_utils import run_bass_kernel_spmd

F32 = mybir.dt.float32
BF16 = mybir.dt.bfloat16
AL = mybir.AluOpType
AF = mybir.ActivationFunctionType


def _register_custom(name, spec_fn):
    import concourse.dve_ops as dops
    from concourse.dve_spec import lower, _has_src1
    from concourse.dve_uop import DveOpSpec

    for o in dops.OPS:
        if o.name == name:
            return o
    spec = spec_fn()
    row = dops._CUSTOM_DVE_ROW_BASE + len(dops.OPS)
    assert row < 0x20, "custom-DVE opcode rows exhausted"
    shas = {}
    for ver in ("v3", "v4"):
        s = DveOpSpec(name=name, opcode=row, uops=lower(spec, ver=ver),
                      rd1_en=_has_src1(spec))
        shas[ver] = s.sha(ver)
    op = dops.DveOp(name, spec, subdim=False, uops_sha=shas)
    dops.OPS.append(op)
    dops.CUSTOM_DVE_SPECS[name] = spec
    dops._SUB_OPCODE_FOR_NAME[name] = row
    return op


def _scale2_spec():
    from concourse.dve_spec import Spec, Src0, Src1, C0, C1
    return Spec(
        body=Src0 * C0 + Src1 * C1,
        reference=lambda in0, in1, s0, s1, imm2: (
            in0.astype(np.float32) * s0 + in1 * s1
        ),
    )


def _dotacc_spec():
    from concourse.dve_spec import Spec, Src0, Src1, AluOp
    return Spec(
        body=Src0 * Src1,
        accum=AluOp.ADD,
        reference=lambda in0, in1, s0, s1, imm2: (
            in0.astype(np.float32) * in1,
            (in0.astype(np.float32) * in1).sum(axis=1, keepdims=True),
        ),
    )


SCALE2_ADD = _register_custom("SCALE2_ADD_ANT", _scale2_spec)
DOT_ACC = _register_custom("DOT_ACC_ANT", _dotacc_spec)

K = 8
C = 512
B = 4
T = 4096
TLOC = T // 2          # output tokens per core
HALO = 256             # lookback halo tokens
NTOK = TLOC + HALO     # 2304 tokens per core slab
NT = NTOK // 128       # 18 partition-tiles
MAIN0 = HALO // 128    # 2: first tile with output tokens
TAU = 1e-6
EPS = 1e-12
# matsf (f32): E_j at j*128 (j=0..6), F_j at (7+j)*128 (j=0..7)  [15 mats]
# matsb (bf16): A_j=I+E_W at j*128, F_j at (7+j)*128 (j=0..6)    [14 mats]
NMF = 15
NMB = 14


def _iget(name, default):
    return int(os.environ.get(name, str(default)))


def _sget(name, default):
    return os.environ.get(name, default)


def _emit(ctx, tc, nc, xd_ap, msk_ap, matsf_ap, matsb_ap, out_ap):
    copy_pat = _sget("COPY_PAT", "AV")      # PSUM->SBUF copy engines (PE lvls)
    red_pat = _sget("RED_PAT", "AAAV")    # nu2 reduce engines
    add_pat = _sget("ADD_PAT", "PPV")        # DMA-level add engines
    pe_mask = _iget("PE_MASK", 0b0000001)  # levels j=0..6 built on PE
    ssp = _iget("STORE_SPLIT", 4)
    pbufs = _iget("PSUM_BUFS", 5)
    sbufs = _iget("SLAB_BUFS", 4)
    nprev = _iget("NPREV_SPLIT", 6)        # col groups per shift DMA

    slab = ctx.enter_context(tc.tile_pool(name="slab", bufs=sbufs))
    sqp = ctx.enter_context(tc.tile_pool(name="sq", bufs=3))
    yp = ctx.enter_context(tc.tile_pool(name="y", bufs=2))
    stp = ctx.enter_context(tc.tile_pool(name="st", bufs=1))
    chp = ctx.enter_context(tc.tile_pool(name="ch", bufs=2))
    pp = ctx.enter_context(tc.psum_pool(name="pp", bufs=2))
    pp2 = ctx.enter_context(tc.psum_pool(name="pp2", bufs=pbufs))

    G = K * NT
    msk_sb = stp.tile([128, 2 * G], F32, tag="msk")     # scm | m1 packed
    matsf = stp.tile([128, NMF * 128], F32, tag="matsf")
    matsb = stp.tile([128, NMB * 128], BF16, tag="matsb")
    nu2 = stp.tile([128, (K + 1) * NT], F32, tag="nu2")  # levels 0..8
    nv2 = stp.tile([128, G], F32, tag="nv2")

    nc.sync.dma_start(out=msk_sb[:, :], in_=msk_ap[:, :])
    for m in range(4):
        c0, c1 = m * 512, min((m + 1) * 512, NMF * 128)
        nc.scalar.dma_start(out=matsf[:, c0:c1], in_=matsf_ap[:, c0:c1])
    nc.sync.dma_start(out=matsb[:, :], in_=matsb_ap[:, :])

    def Amat(j):
        return matsb[:, j * 128:(j + 1) * 128]

    def Fmatb(j):
        return matsb[:, (7 + j) * 128:(8 + j) * 128]

    def Emat(j):
        return matsf[:, j * 128:(j + 1) * 128]

    def Fmatf(j):
        return matsf[:, (7 + j) * 128:(8 + j) * 128]

    S = {}
    S[0] = slab.tile([128, NT * C], BF16, tag="S", name="S0")
    xqs = [nc.sync, nc.scalar, nc.gpsimd]
    nxq = _iget("XQ", 3)
    for i in range(NT):
        xqs[i % nxq].dma_start(out=S[0][:, i * C:(i + 1) * C],
                               in_=xd_ap[i * 128:(i + 1) * 128, :])

    # zero the stat columns never written by reduces (halo cols + col 0)
    nc.gpsimd.memset(nu2[:, :], 0.0)

    ncop = [0]
    nred = [0]
    nadd = [0]

    def _pat(pat, ctr):
        e = pat[ctr[0] % len(pat)]
        ctr[0] += 1
        return e

    def reduce_into(dst, src0, src1, eng):
        """dst[128,1] = sum_f src0*src1. "A": ACT Square+accum (requires
        src0==src1; PSUM or SBUF src). "V": custom DVE dot-accum (SBUF)."""
        sq = sqp.tile([128, C], BF16, tag="sq")
        if eng == "A":
            nc.scalar.activation(sq[:, :], src0, AF.Square, accum_out=dst)
        else:
            nc.vector._custom_dve(DOT_ACC, out=sq[:, :], accum_out=dst,
                                  in0=src0, in1=src1)

    def copy_into(dst, src, eng):
        if eng == "A":
            nc.scalar.activation(dst, src, AF.Copy)
        else:
            nc.vector.tensor_copy(dst, src)

    # nu2_0: ACT squares reading x directly (ACT is idle during x load)
    for i in range(1, NT):
        xs = S[0][:, i * C:(i + 1) * C]
        reduce_into(nu2[:, i:i + 1], xs, xs,
                    _sget("NU0_PAT", "AV")[i % 2])

    def build_pe(jb, Sj, Sn):
        j = jb - 1
        for i in range(NT):
            ps = pp2.tile([128, C], F32, tag="psb", name="psb")
            rj = Sj[:, i * C:(i + 1) * C]
            if i == 0:
                nc.tensor.matmul(ps[:, :], Amat(j), rj, start=True, stop=True)
            else:
                nc.tensor.matmul(ps[:, :], Amat(j), rj, start=True, stop=False)
                nc.tensor.matmul(ps[:, :], Fmatb(j),
                                 Sj[:, (i - 1) * C:i * C],
                                 start=False, stop=True)
            re = _pat(red_pat, nred) if i >= 1 else None
            if re == "A":
                reduce_into(nu2[:, jb * NT + i:jb * NT + i + 1],
                            ps[:, :], ps[:, :], "A")
            copy_into(Sn[:, i * C:(i + 1) * C], ps[:, :], _pat(copy_pat, ncop))
            if re == "V":
                sb = Sn[:, i * C:(i + 1) * C]
                reduce_into(nu2[:, jb * NT + i:jb * NT + i + 1], sb, sb, "V")

    def build_dma(jb, Sj, Sn):
        """shift-by-W via SBUF->SBUF DMA into Sn, then Sn += S_j in place."""
        W = 1 << (jb - 1)
        gs = NT // nprev
        for g in range(nprev):
            c0, c1 = g * gs, (g + 1) * gs
            engs = (nc.sync, nc.scalar)
            engs[g % 2].dma_start(
                out=Sn[W:128, c0 * C:c1 * C],
                in_=Sj[0:128 - W, c0 * C:c1 * C],
            )
            lo = max(c0, 1)
            if lo < c1:
                engs[(g + 1) % 2].dma_start(
                    out=Sn[0:W, lo * C:c1 * C],
                    in_=Sj[128 - W:128, (lo - 1) * C:(c1 - 1) * C],
                )
        nc.gpsimd.memset(Sn[0:W, 0:C], 0.0)
        for i in range(NT):
            dst = Sn[:, i * C:(i + 1) * C]
            src = Sj[:, i * C:(i + 1) * C]
            ae = _pat(add_pat, nadd)
            if ae == "P":
                nc.gpsimd.tensor_add(dst, src, dst)
            else:
                nc.vector.tensor_add(dst, src, dst)
            re = _pat(red_pat, nred) if i >= 1 else None
            if re is not None:
                reduce_into(nu2[:, jb * NT + i:jb * NT + i + 1],
                            dst, dst, re)

    def build(jb, Pp=None):
        j = jb - 1
        Sj = S[j]
        Sn = slab.tile([128, NT * C], BF16, tag="S", name=f"S{jb}")
        S[jb] = Sn
        if (pe_mask >> j) & 1:
            build_pe(jb, Sj, Sn)
        else:
            build_dma(jb, Sj, Sn)
        if Pp is not None:
            for i in range(MAIN0, NT):
                reduce_into(Pp[:, i:i + 1],
                            Sn[:, i * C:(i + 1) * C],
                            Sn[:, (i - 1) * C:i * C], "V")

    def stat_shift(j):
        """nv2_j = shift_{2^j}(nu2_j) via PE matmuls (fp32, 18 cols)."""
        nuj = nu2[:, j * NT:(j + 1) * NT]
        dst = nv2[:, j * NT:(j + 1) * NT]
        ps = pp.tile([128, NT], F32, tag="ps")
        if j == K - 1:
            # W=128: whole-tile column shift; F_7 = I. Column 0 of dst is
            # zeroed in SBUF (Pool cannot touch PSUM).
            nc.tensor.matmul(ps[:, 1:NT], Fmatf(j), nuj[:, 0:NT - 1],
                             start=True, stop=True)
            nc.vector.tensor_copy(dst[:, 1:NT], ps[:, 1:NT])
            nc.gpsimd.memset(dst[:, 0:1], 0.0)
        else:
            nc.tensor.matmul(ps[:, :], Emat(j), nuj, start=True, stop=False)
            nc.tensor.matmul(ps[:, 1:NT], Fmatf(j), nuj[:, 0:NT - 1],
                             start=False, stop=True, skip_group_check=True)
            nc.vector.tensor_copy(dst[:, :], ps[:, :])

    def t(tag):
        return chp.tile([128, NT], F32, tag=tag, name=tag)

    chain_st = {}

    def chain_part1(j):
        """rnu/rnv for scale j — depends only on nu2_j / nv2_j, so it can be
        emitted before the level j+1 build to keep the ACT sqrts early."""
        n = nu2[:, j * NT:(j + 1) * NT]
        v = nv2[:, j * NT:(j + 1) * NT]
        rnu, rnv = t("rnu"), t("rnv")
        nc.scalar.activation(rnu[:, :], n, AF.Sqrt)
        nc.vector.tensor_scalar(out=rnu[:, :], in0=rnu[:, :], scalar1=EPS,
                                scalar2=None, op0=AL.max)
        nc.vector.reciprocal(rnu[:, :], rnu[:, :])
        nc.scalar.activation(rnv[:, :], v, AF.Sqrt)
        nc.vector.tensor_scalar(out=rnv[:, :], in0=rnv[:, :], scalar1=EPS,
                                scalar2=None, op0=AL.max)
        nc.vector.reciprocal(rnv[:, :], rnv[:, :])
        chain_st[j] = (rnu, rnv)

    def chain_and_y(j, shift_col):
        """Scalar chain for scale j + y tiles + stores.

        shift_col: if True, y reads prev = column-shifted S_j view (j=7)."""
        n = nu2[:, j * NT:(j + 1) * NT]
        np1 = nu2[:, (j + 1) * NT:(j + 2) * NT]
        v = nv2[:, j * NT:(j + 1) * NT]
        scm = msk_sb[:, j * NT:(j + 1) * NT]
        m1 = msk_sb[:, G + j * NT:G + (j + 1) * NT]

        rnu, rnv = chain_st.pop(j)
        P, cc, at, bt, rd = (t(x) for x in ("P", "cc", "at", "bt", "rd"))
        # P here holds 2P = np1 - n - v; the 0.5 folds into STT fusions.
        nc.vector.tensor_sub(P[:, :], np1, n)
        nc.vector.tensor_sub(P[:, :], P[:, :], v)
        nc.vector.tensor_mul(cc[:, :], P[:, :], rnu[:, :])
        nc.vector.scalar_tensor_tensor(out=cc[:, :], in0=cc[:, :],
                                       scalar=0.5, in1=rnv[:, :],
                                       op0=AL.mult, op1=AL.mult)
        nc.vector.scalar_tensor_tensor(out=at[:, :], in0=P[:, :],
                                       scalar=0.5, in1=v,
                                       op0=AL.mult, op1=AL.subtract)
        nc.vector.tensor_mul(at[:, :], at[:, :], rnv[:, :])
        nc.vector.scalar_tensor_tensor(out=bt[:, :], in0=P[:, :],
                                       scalar=-0.5, in1=n,
                                       op0=AL.mult, op1=AL.add)
        nc.vector.tensor_mul(bt[:, :], bt[:, :], rnu[:, :])
        nc.vector.tensor_scalar(out=rd[:, :], in0=cc[:, :], scalar1=1.0,
                                scalar2=TAU, op0=AL.add, op1=AL.max)
        nc.vector.reciprocal(rd[:, :], rd[:, :])
        tA, tB, s0, s1 = (t(x) for x in ("tA", "tB", "s0", "s1"))
        nc.vector.tensor_mul(tA[:, :], at[:, :], cc[:, :])
        nc.vector.tensor_sub(tA[:, :], tA[:, :], bt[:, :])
        nc.vector.tensor_mul(tA[:, :], tA[:, :], rd[:, :])
        nc.vector.tensor_sub(tA[:, :], tA[:, :], at[:, :])
        nc.vector.tensor_mul(tA[:, :], tA[:, :], rnu[:, :])
        nc.vector.tensor_mul(tB[:, :], bt[:, :], cc[:, :])
        nc.vector.tensor_sub(tB[:, :], tB[:, :], at[:, :])
        nc.vector.tensor_mul(tB[:, :], tB[:, :], rd[:, :])
        nc.vector.tensor_add(tB[:, :], tB[:, :], bt[:, :])
        nc.vector.tensor_mul(tB[:, :], tB[:, :], rnv[:, :])
        # y = s0*S_j + s1*prev_src; prev = S_{j+1} - S_j:
        # s0 = (tA-tB+2)*scm + m1 ; s1 = (tB-1)*scm
        # For shift_col (j=7): prev_src = shift_128(S_j) directly, so
        # s0' = s0 + s1 = (tA+1)*scm + m1, s1' = s1.
        nc.vector.tensor_sub(s0[:, :], tA[:, :], tB[:, :])
        nc.vector.scalar_tensor_tensor(out=s0[:, :], in0=s0[:, :], scalar=2.0,
                                       in1=scm, op0=AL.add, op1=AL.mult)
        nc.vector.tensor_add(s0[:, :], s0[:, :], m1)
        nc.vector.scalar_tensor_tensor(out=s1[:, :], in0=tB[:, :], scalar=1.0,
                                       in1=scm, op0=AL.subtract, op1=AL.mult)
        if shift_col:
            nc.vector.tensor_add(s0[:, :], s0[:, :], s1[:, :])

        Y = yp.tile([128, (NT - MAIN0) * C], BF16, tag="Y", name="Y")
        for i in range(MAIN0, NT):
            ic = (i - MAIN0) * C
            in1 = (S[j][:, (i - 1) * C:i * C] if shift_col
                   else S[j + 1][:, i * C:(i + 1) * C])
            nc.vector._custom_dve(
                SCALE2_ADD, out=Y[:, ic:ic + C],
                in0=S[j][:, i * C:(i + 1) * C],
                in1=in1,
                s0=s0[:, i:i + 1],
                s1=s1[:, i:i + 1],
            )
        w = (NT - MAIN0) * C // ssp
        for q in range(ssp):
            nc.sync.dma_start(out=out_ap[j, :, q * w:(q + 1) * w],
                              in_=Y[:, q * w:(q + 1) * w])

    for jb in range(1, K):
        stat_shift(jb - 1)
        chain_part1(jb - 1)
        build(jb)
        chain_and_y(jb - 1, False)

    # ---- level 7 tail: S_8[i] = S_7[i] + S_7[i-1] is a free column-shift
    # pair, materialized by Pool adds (Pool and ACT are idle at the tail);
    # nu2_8 from ACT squares of S_8. Only tiles >= 1 are needed. ----
    j = K - 1
    stat_shift(j)
    chain_part1(j)
    s8_pat = _sget("S8_PAT", "P")
    Sn8 = slab.tile([128, NT * C], BF16, tag="S", name="S8")
    S[K] = Sn8
    for i in range(1, NT):
        dst = Sn8[:, i * C:(i + 1) * C]
        ae = s8_pat[i % len(s8_pat)]
        if ae == "P":
            nc.gpsimd.tensor_add(dst, S[j][:, i * C:(i + 1) * C],
                                 S[j][:, (i - 1) * C:i * C])
        else:
            nc.vector.tensor_add(dst, S[j][:, i * C:(i + 1) * C],
                                 S[j][:, (i - 1) * C:i * C])
        reduce_into(nu2[:, K * NT + i:K * NT + i + 1], dst, dst,
                    _sget("SQ8_PAT", "A"))
    chain_and_y(j, False)


_PROG = None


def _program():
    global _PROG
    if _PROG is None:
        nc = bacc.Bacc(
            "TRN2", target_bir_lowering=False, debug=False, num_devices=8
        )
        xd_ap = nc.dram_tensor("x", [NTOK, C], BF16, kind="ExternalInput").ap()
        msk_ap = nc.dram_tensor("msk", [128, 2 * K * NT], F32,
                                kind="ExternalInput").ap()
        matsf_ap = nc.dram_tensor("matsf", [128, NMF * 128], F32,
                                  kind="ExternalInput").ap()
        matsb_ap = nc.dram_tensor("matsb", [128, NMB * 128], BF16,
                                  kind="ExternalInput").ap()
        out_ap = nc.dram_tensor(
            "out", [K, 128, (NT - MAIN0) * C], BF16, kind="ExternalOutput"
        ).ap()
        with tile.TileContext(nc) as tc:
            with ExitStack() as ctx:
                _emit(ctx, tc, nc, xd_ap, msk_ap, matsf_ap, matsb_ap, out_ap)
        nc.compile()
        _PROG = nc
    return _PROG


def _host_consts(h):
    """Packed scm|m1 [128, 2*K*NT] plus f32 stat mats and bf16 build mats.

    Token (p, col i) = local slab index i*128+p, global g = h*TLOC-HALO+that.
    scm = 2^-j * (g >= 2W-1); m1 = 2^-j * (W <= g < 2W-1).
    """
    G = K * NT
    msk = np.zeros((128, 2 * G), np.float32)
    g0 = h * TLOC - HALO
    loc = np.arange(NTOK).reshape(NT, 128).T  # [128, NT]
    g = g0 + loc
    for j in range(K):
        W = 1 << j
        sc = 2.0 ** (-j)
        msk[:, j * NT:(j + 1) * NT] = sc * (g >= 2 * W - 1)
        msk[:, G + j * NT:G + (j + 1) * NT] = sc * ((g >= W) & (g < 2 * W - 1))

    matsf = np.zeros((128, NMF * 128), np.float32)
    matsb = np.zeros((128, NMB * 128), np.float32)
    for j in range(K - 1):
        W = 1 << j
        matsf[:, j * 128:(j + 1) * 128] = np.eye(128, 128, W)
        matsb[:, j * 128:(j + 1) * 128] = np.eye(128) + np.eye(128, 128, W)
        matsb[:, (7 + j) * 128:(8 + j) * 128] = np.eye(128, 128, -(128 - W))
    for j in range(K):
        W = 1 << j
        matsf[:, (7 + j) * 128:(8 + j) * 128] = np.eye(128, 128, -(128 - W))
    return msk, matsf, matsb


def make_in_maps(x):
    import ml_dtypes
    x = np.ascontiguousarray(np.asarray(x, np.float32))
    in_maps = []
    consts = [_host_consts(h) for h in range(2)]
    for core in range(8):
        b, h = divmod(core, 2)
        slab = np.zeros((NTOK, C), np.float32)
        if h == 0:
            slab[HALO:] = x[b, :TLOC]
        else:
            slab[:] = x[b, TLOC - HALO:T]
        msk, matsf, matsb = consts[h]
        in_maps.append({
            "x": slab.astype(ml_dtypes.bfloat16),
            "msk": msk,
            "matsf": matsf,
            "matsb": matsb.astype(ml_dtypes.bfloat16),
        })
    return in_maps


def assemble(results):
    out = np.empty((B, T, K, C), np.float32)
    for core in range(8):
        b, h = divmod(core, 2)
        a = np.asarray(results[core]["out"]).astype(np.float32)
        # [K, 128, 16*C]: token t = i*128+p at (j, p, i*C+c)
        a = a.reshape(K, 128, NT - MAIN0, C).transpose(2, 1, 0, 3)
        out[b, h * TLOC:(h + 1) * TLOC] = a.reshape(TLOC, K, C)
    return out


def kernel(x):
    nc = _program()
    res = run_bass_kernel_spmd(nc, make_in_maps(x), list(range(8)))
    return assemble(res.results)
